# revision 4
# baseline (speedup 1.0000x reference)
"""BitMultiheadAttention (1.58-bit, inference) on 8 Trainium2 NeuronCores.

The metric for this problem is warm wall-clock of `kernel(**inputs)`, which
is dominated by host<->device transfer over the axon tunnel (~50 MB/s in,
~25 MB/s out).  The design therefore minimizes shipped bytes:

  - activations are quantized to int8 on the host (the reference quantizes
    them to 8 bits anyway: qx = clip(round(x*128/gamma), -128, 127)) and
    shipped pre-transposed [E, tokens] together with tiny per-token scale
    rows; 96 MB fp32 -> 24 MB int8.
  - ternary weights are packed 4-per-byte (base-27: 27*w0+9*w1+3*w2+w3,
    values in [-40, 40]) and unpacked on device with a few DVE ops;
    64 MB fp16 -> 8 MB.  Weight-derived device arrays are cached keyed on
    a fingerprint, so repeat calls with the same weights ship nothing.
  - the output is quantized on device to int8 with a per-token scale
    (error <= rowmax/254, far under the 2e-2 tolerance); 32 MB -> 8 MB.
  - the donated output buffers are created on device (the stock runner
    ships 32 MB of host zeros per call).

Sharding: core c -> batch b = c//2, query-token half = c%2.  key/value of
the batch are replicated to both cores of a pair; no collectives.

Per-core device pipeline (all matmuls fp16 operands, fp32 PSUM):
  1. unpack ternary weights: cast-DMA packed int8 -> fp16, peel base-27
     digits with round(x/b) via the fp16 magic-number trick.
  2. K^T/Q^T computed directly in [e, t] layout: psum[e,t] = Wk @ qx^T,
     dequant = psum * (ws*gamma_t/128) (broadcast row) + bias (per-e col).
     1/sqrt(D) and bias/sqrt(D) are folded into the Q scales on host.
  3. V in [t, e] stride-66 per-head layout (64 data + ones column which
     produces the softmax denominator): psum[t,e] = qx^T.T @ Wv, dequant
     via per-token activation scale + bias row.
  4. attention per head pair exactly as the fp16 flash-style original:
     S^T[k,q] = K^T.T @ Q^T, exp on ACT (scores are O(1), no max needed),
     ctx^T[d,q] accumulated over k-chunks with the ones-row denominator.
  5. softmax normalization, transpose to [t, e], reference-style 8-bit
     re-quantization, out-projection, per-token int8 output quant.
"""

import sys
import zlib
import functools
from contextlib import ExitStack

for _p in ("/opt/trn_rl_repo",):
    if _p not in sys.path:
        sys.path.insert(0, _p)

import numpy as np
import jax
import jax.numpy as jnp

import concourse.bass as bass
import concourse.tile as tile
from concourse import mybir
from concourse.bass2jax import (_bass_exec_p, install_neuronx_cc_hook,
                                partition_id_tensor)
from jax.experimental.shard_map import shard_map
from jax.sharding import Mesh, PartitionSpec, NamedSharding

P = 128
B, L, E, H, D = 4, 2048, 1024, 16, 64
NCORES = 8
LQ = L // 2
EPS = 1e-5
QF = 128.0
MAGIC = 1536.0
SQRTD = 8.0
OQ = 127.0  # output shipping quant range
F32 = mybir.dt.float32
F16 = mybir.dt.float16
I8 = mybir.dt.int8
AX = mybir.AxisListType.X
OP = mybir.AluOpType
EXP = mybir.ActivationFunctionType.Exp
COPY = mybir.ActivationFunctionType.Copy

VSTRIDE = 66
TK = L // P    # 16 k/v token tiles
TQ = LQ // P   # 8 q token tiles
EC = E // P    # 8 chunks of embedding dim


# ---------------------------------------------------------------- device ----

def _emit(ctx: ExitStack, tc: tile.TileContext, io: dict, os_imm: float):
    nc = tc.nc

    res = ctx.enter_context(tc.tile_pool(name="res", bufs=1))
    kT = [res.tile([P, L], F16, tag=f"kT{c}", name=f"kT{c}") for c in range(EC)]
    qT = [res.tile([P, LQ], F16, tag=f"qT{c}", name=f"qT{c}") for c in range(EC)]
    vres = [res.tile([P, H * VSTRIDE], F16, tag=f"v{t}", name=f"v{t}")
            for t in range(TK)]
    ctxT = [res.tile([P, E], F16, tag=f"ctxT{t}", name=f"ctxT{t}")
            for t in range(TQ)]
    # per-e-chunk bias columns for K/Q (f32 [128, EC])
    bcol = res.tile([P, 2 * EC], F32, tag="bcol", name="bcol")
    nc.gpsimd.dma_start(bcol[:, 0:EC], io["kb"][:])
    nc.gpsimd.dma_start(bcol[:, EC:2 * EC], io["qb"][:])
    gvt = res.tile([P, TK], F32, tag="gvt", name="gvt")
    nc.gpsimd.dma_start(gvt[:], io["gv"][:])

    dram = ctx.enter_context(tc.tile_pool(name="dram", bufs=1, space="DRAM"))
    rs_dram = dram.tile([H, LQ], F32, tag="rs", name="rs")
    cn_dram = [dram.tile([64, LQ], F16, tag=f"cnd{h}", name=f"cnd{h}")
               for h in range(H)]
    qn_dram = dram.tile([LQ, E], F16, tag="qnd", name="qnd")

    for t in range(TK):
        ones_ap = vres[t][:].rearrange("p (h c) -> p h c", c=VSTRIDE)[:, :, 64:65]
        nc.vector.memset(ones_ap, 1.0)

    def unpack_weights(stk: ExitStack, name, wdram):
        """Packed base-27 ternary [E, 256] int8 -> 8 fp16 tiles [128, E]."""
        sp = stk.enter_context(tc.tile_pool(name=f"ws_{name}", bufs=2))
        tp = stk.enter_context(tc.tile_pool(name=f"wt_{name}", bufs=4))
        wp = stk.enter_context(tc.tile_pool(name=f"w_{name}", bufs=1))
        w16 = [wp.tile([P, E], F16, tag=f"{name}{i}", name=f"{name}{i}")
               for i in range(EC)]
        for i in range(EC):
            pk = sp.tile([P, 256], F16, tag="pk", name="pk")
            nc.gpsimd.dma_start(pk[:], wdram[i * P:(i + 1) * P, :])
            rem = pk
            for lvl, base in ((0, 27.0), (1, 9.0), (2, 3.0)):
                q = w16[i][:, lvl * 256:(lvl + 1) * 256]
                d = tp.tile([P, 256], F16, tag="d", name="d")
                nc.vector.tensor_scalar(d[:], rem[:], 1.0 / base, MAGIC,
                                        OP.mult, OP.add)
                nc.vector.tensor_scalar(q, d[:], -MAGIC, None, OP.add)
                dst = (tp.tile([P, 256], F16, tag="r", name="r")[:]
                       if lvl < 2 else w16[i][:, 768:1024])
                nc.vector.scalar_tensor_tensor(dst, q, -base, rem[:],
                                               OP.mult, OP.add)
                rem = dst
        return w16

    def kq_phase(stk, name, wdram, xdram, grow_dram, ntok, out_T, bias_off):
        """out_T[e, t] = W @ qx^T, dequant via broadcast gamma row + bias col."""
        w16 = unpack_weights(stk, name, wdram)
        xp = stk.enter_context(tc.tile_pool(name=f"x_{name}", bufs=1))
        x16 = [xp.tile([P, ntok], F16, tag=f"x{i}", name=f"x{i}")
               for i in range(EC)]
        for i in range(EC):
            nc.gpsimd.dma_start(x16[i][:], xdram[i * P:(i + 1) * P, :])
        gb = xp.tile([P, ntok], F32, tag="gb", name="gb")
        nc.gpsimd.dma_start(gb[:], grow_dram[0:1, :].to_broadcast((P, ntok)))
        pp = stk.enter_context(tc.tile_pool(name=f"ps_{name}", bufs=4,
                                            space="PSUM"))
        tp = stk.enter_context(tc.tile_pool(name=f"t_{name}", bufs=4))
        for e in range(EC):
            for ts in range(ntok // 512):
                ps = pp.tile([P, 512], F32, tag="ps", name="ps")
                for i in range(EC):
                    nc.tensor.matmul(ps[:],
                                     lhsT=w16[i][:, e * P:(e + 1) * P],
                                     rhs=x16[i][:, ts * 512:(ts + 1) * 512],
                                     start=(i == 0), stop=(i == EC - 1))
                tmp = tp.tile([P, 512], F32, tag="tmp", name="tmp")
                nc.vector.tensor_tensor(tmp[:], ps[:],
                                        gb[:, ts * 512:(ts + 1) * 512],
                                        op=OP.mult)
                nc.vector.tensor_scalar(out_T[e][:, ts * 512:(ts + 1) * 512],
                                        tmp[:], bcol[:, bias_off + e:bias_off + e + 1],
                                        None, OP.add)

    # --- K^T, Q^T ---
    with ExitStack() as stk:
        kq_phase(stk, "k", io["wk"], io["kT"], io["gk"], L, kT, 0)
    with ExitStack() as stk:
        kq_phase(stk, "q", io["wq"], io["qT"], io["gq"], LQ, qT, EC)

    # --- V (dequant straight into the stride-66 per-head layout) ---
    with ExitStack() as stk:
        wv16 = unpack_weights(stk, "v", io["wv"])
        xp = stk.enter_context(tc.tile_pool(name="x_v", bufs=1))
        vx16 = [xp.tile([P, L], F16, tag=f"vx{i}", name=f"vx{i}")
                for i in range(EC)]
        for i in range(EC):
            nc.gpsimd.dma_start(vx16[i][:], io["vT"][i * P:(i + 1) * P, :])
        vbb = xp.tile([P, E], F16, tag="vbb", name="vbb")
        nc.gpsimd.dma_start(vbb[:], io["vb"][0:1, :].to_broadcast((P, E)))
        pp = stk.enter_context(tc.tile_pool(name="ps_v", bufs=4, space="PSUM"))
        tp = stk.enter_context(tc.tile_pool(name="t_v", bufs=4))
        for tt in range(TK):
            for eh in range(2):
                ps = pp.tile([P, 512], F32, tag="ps", name="ps")
                for i in range(EC):
                    nc.tensor.matmul(ps[:],
                                     lhsT=vx16[i][:, tt * P:(tt + 1) * P],
                                     rhs=wv16[i][:, eh * 512:(eh + 1) * 512],
                                     start=(i == 0), stop=(i == EC - 1))
                tmp = tp.tile([P, 512], F16, tag="tmp", name="tmp")
                nc.scalar.activation(tmp[:], ps[:], COPY,
                                     scale=gvt[:, tt:tt + 1])
                out_ap = (vres[tt][:, eh * 8 * VSTRIDE:(eh * 8 + 8) * VSTRIDE]
                          .rearrange("p (h c) -> p h c", c=VSTRIDE)[:, :, 0:64])
                nc.vector.tensor_tensor(out_ap, tmp[:],
                                        vbb[:, eh * 512:(eh + 1) * 512],
                                        op=OP.add)

    # ---------------- attention ----------------
    with ExitStack() as stk:
        sp = stk.enter_context(tc.tile_pool(name="spsum", bufs=2, space="PSUM"))
        cp = stk.enter_context(tc.tile_pool(name="cpsum", bufs=1, space="PSUM"))
        ptp = stk.enter_context(tc.tile_pool(name="pt", bufs=3))
        c65p = stk.enter_context(tc.tile_pool(name="c65", bufs=4))
        cnp = stk.enter_context(tc.tile_pool(name="cn", bufs=4))
        rsp = stk.enter_context(tc.tile_pool(name="rsbc", bufs=3))

        for hp in range(H // 2):
            ctx_ps = {}
            for hh in range(2):
                for qc in range(2):
                    ctx_ps[(hh, qc)] = cp.tile([65, 512], F32, tag=f"c{hh}{qc}",
                                               name=f"c{hh}{qc}")
            for kc in range(TK):
                for hh in range(2):
                    h = 2 * hp + hh
                    s_ps = sp.tile([P, LQ], F32, tag="s", name="s")
                    for qc in range(2):
                        nc.tensor.matmul(
                            s_ps[:, qc * 512:(qc + 1) * 512],
                            lhsT=kT[hp][hh * 64:(hh + 1) * 64,
                                        kc * P:(kc + 1) * P],
                            rhs=qT[hp][hh * 64:(hh + 1) * 64,
                                       qc * 512:(qc + 1) * 512],
                            start=True, stop=True)
                    pt = ptp.tile([P, LQ], F16, tag="pt", name="pt")
                    nc.scalar.activation(pt[:], s_ps[:], EXP)
                    for qc in range(2):
                        nc.tensor.matmul(
                            ctx_ps[(hh, qc)][:],
                            lhsT=vres[kc][:, h * VSTRIDE:h * VSTRIDE + 65],
                            rhs=pt[:, qc * 512:(qc + 1) * 512],
                            start=(kc == 0), stop=(kc == TK - 1))
            for hh in range(2):
                h = 2 * hp + hh
                c65 = c65p.tile([65, LQ], F32, tag="c65", name="c65")
                for qc in range(2):
                    nc.vector.tensor_copy(c65[:, qc * 512:(qc + 1) * 512],
                                          ctx_ps[(hh, qc)][:])
                nc.vector.reciprocal(c65[64:65, :], c65[64:65, :])
                nc.sync.dma_start(rs_dram[h:h + 1, :], c65[64:65, :])
                rst = rsp.tile([64, LQ], F32, tag="rst", name="rst")
                nc.gpsimd.dma_start(rst[:],
                                    rs_dram[h:h + 1, :].to_broadcast((64, LQ)))
                cn = cnp.tile([64, LQ], F16, tag="cn", name="cn")
                nc.vector.tensor_tensor(cn[:], c65[0:64, :], rst[:], op=OP.mult)
                nc.gpsimd.dma_start(cn_dram[h][:], cn[:])
                for tt in range(TQ):
                    nc.sync.dma_start_transpose(
                        ctxT[tt][:, h * 64:(h + 1) * 64],
                        cn_dram[h][:, tt * P:(tt + 1) * P])

    # ---------------- out-projection ----------------
    with ExitStack() as stk:
        wo16 = unpack_weights(stk, "o", io["wo"])
        smp = stk.enter_context(tc.tile_pool(name="smalls", bufs=6))
        qnp = stk.enter_context(tc.tile_pool(name="qn", bufs=3))
        qcp = stk.enter_context(tc.tile_pool(name="qctx", bufs=1))
        opp = stk.enter_context(tc.tile_pool(name="ops", bufs=4, space="PSUM"))
        outp = stk.enter_context(tc.tile_pool(name="out", bufs=3))
        ogp = stk.enter_context(tc.tile_pool(name="og", bufs=1))

        obb = qcp.tile([P, E], F32, tag="obb", name="obb")
        nc.gpsimd.dma_start(obb[:], io["ob"][0:1, :].to_broadcast((P, E)))
        og_acc = ogp.tile([P, TQ], F32, tag="oga", name="oga")

        qctxT = [qcp.tile([P, LQ], F16, tag=f"qc{c}", name=f"qc{c}")
                 for c in range(EC)]
        d2cols = []
        for tt in range(TQ):
            g = smp.tile([P, 1], F32, tag="g", name="g")
            nc.vector.tensor_reduce(g[:], ctxT[tt][:], axis=AX, op=OP.max,
                                    apply_absolute_value=True)
            nc.vector.tensor_scalar_max(g[:], g[:], EPS)
            s2 = smp.tile([P, 1], F32, tag="s2", name="s2")
            nc.vector.reciprocal(s2[:], g[:])
            nc.vector.tensor_scalar_mul(s2[:], s2[:], QF)
            d2 = smp.tile([P, 1], F32, tag="d2", name="d2")
            nc.vector.tensor_scalar_mul(d2[:], g[:], os_imm / QF)
            d2cols.append(d2)

            qm = qnp.tile([P, E], F16, tag="qm", name="qm")
            nc.vector.tensor_scalar(qm[:], ctxT[tt][:], s2[:], MAGIC,
                                    OP.mult, OP.add)
            qn = qnp.tile([P, E], F16, tag="qnt", name="qnt")
            nc.vector.tensor_scalar(qn[:], qm[:], -MAGIC, QF - 1.0,
                                    OP.add, OP.min)
            nc.gpsimd.dma_start(qn_dram[tt * P:(tt + 1) * P, :], qn[:])
            for c in range(EC):
                nc.sync.dma_start_transpose(
                    qctxT[c][:, tt * P:(tt + 1) * P],
                    qn_dram[tt * P:(tt + 1) * P, c * P:(c + 1) * P])

        for tt in range(TQ):
            ot = outp.tile([P, E], F32, tag="ot", name="ot")
            for e in range(2):
                ps = opp.tile([P, 512], F32, tag="ops", name="ops")
                for c in range(EC):
                    nc.tensor.matmul(ps[:],
                                     lhsT=qctxT[c][:, tt * P:(tt + 1) * P],
                                     rhs=wo16[c][:, e * 512:(e + 1) * 512],
                                     start=(c == 0), stop=(c == EC - 1))
                sl = ot[:, e * 512:(e + 1) * 512]
                nc.scalar.activation(sl, ps[:], COPY, scale=d2cols[tt][:])
                nc.vector.tensor_tensor(sl, sl,
                                        obb[:, e * 512:(e + 1) * 512],
                                        op=OP.add)
            # int8 shipping quant: per-token scale = rowmax/127
            go = smp.tile([P, 1], F32, tag="go", name="go")
            nc.vector.tensor_reduce(go[:], ot[:], axis=AX, op=OP.max,
                                    apply_absolute_value=True)
            nc.vector.tensor_scalar_max(go[:], go[:], 1e-30)
            nc.vector.tensor_copy(og_acc[:, tt:tt + 1], go[:])
            ro = smp.tile([P, 1], F32, tag="ro", name="ro")
            nc.vector.reciprocal(ro[:], go[:])
            nc.vector.tensor_scalar_mul(ro[:], ro[:], OQ)
            o16 = outp.tile([P, E], F16, tag="o16", name="o16")
            nc.vector.tensor_scalar(o16[:], ot[:], ro[:], MAGIC,
                                    OP.mult, OP.add)
            o16b = outp.tile([P, E], F16, tag="o16b", name="o16b")
            nc.vector.tensor_scalar(o16b[:], o16[:], -MAGIC, None, OP.add)
            nc.gpsimd.dma_start(io["oq"][tt * P:(tt + 1) * P, :], o16b[:])
        nc.sync.dma_start(io["og"][:], og_acc[:])


def _hoist_excess_waits(nc: bass.Bass):
    """Walrus encodes at most 1 semaphore wait on a DMA DIRECT2D / NoOp and 2
    on compute instruction structs.  Hoist excess waits onto NoOp instructions
    inserted just before the offender on the same engine."""
    import bass_rust
    nwh = 0
    for blk in nc.m.functions[0].blocks:
        insts = blk.instructions
        i = 0
        while i < len(insts):
            ins = insts[i]
            si = ins.sync_info
            limit = 1
            if si is not None and si.on_wait and len(si.on_wait) > limit:
                ow = list(si.on_wait)
                ins.sync_info = bass_rust.SyncInfo(
                    on_wait=[], on_update=list(si.on_update))
                pos = i
                for j in range(len(ow)):
                    nop = mybir.InstNoOp(name=f"WH{nwh}-{ins.name}",
                                         ins=[], outs=[])
                    nop.engine = ins.engine
                    nop.sync_info = bass_rust.SyncInfo(
                        on_wait=[ow[j]], on_update=[])
                    insts.insert(pos, nop)
                    pos += 1
                    nwh += 1
                i = pos + 1
            else:
                i += 1
    return nwh


def _build(os_imm: float) -> bass.Bass:
    nc = bass.Bass(trn_type="TRN2", num_swdge_queues=4)
    io = {
        "qT": nc.dram_tensor("qT", [E, LQ], I8, kind="ExternalInput"),
        "kT": nc.dram_tensor("kT", [E, L], I8, kind="ExternalInput"),
        "vT": nc.dram_tensor("vT", [E, L], I8, kind="ExternalInput"),
        "wq": nc.dram_tensor("wq", [E, 256], I8, kind="ExternalInput"),
        "wk": nc.dram_tensor("wk", [E, 256], I8, kind="ExternalInput"),
        "wv": nc.dram_tensor("wv", [E, 256], I8, kind="ExternalInput"),
        "wo": nc.dram_tensor("wo", [E, 256], I8, kind="ExternalInput"),
        "gq": nc.dram_tensor("gq", [1, LQ], F32, kind="ExternalInput"),
        "gk": nc.dram_tensor("gk", [1, L], F32, kind="ExternalInput"),
        "gv": nc.dram_tensor("gv", [P, TK], F32, kind="ExternalInput"),
        "qb": nc.dram_tensor("qb", [P, EC], F32, kind="ExternalInput"),
        "kb": nc.dram_tensor("kb", [P, EC], F32, kind="ExternalInput"),
        "vb": nc.dram_tensor("vb", [1, E], F16, kind="ExternalInput"),
        "ob": nc.dram_tensor("ob", [1, E], F32, kind="ExternalInput"),
        "oq": nc.dram_tensor("oq", [LQ, E], I8, kind="ExternalOutput"),
        "og": nc.dram_tensor("og", [P, TQ], F32, kind="ExternalOutput"),
    }
    io = {k: v[:] for k, v in io.items()}
    with ExitStack() as ctx:
        tc = ctx.enter_context(tile.TileContext(nc))
        _emit(ctx, tc, io, os_imm)
    _hoist_excess_waits(nc)
    nc.finalize()
    return nc


# ---------------------------------------------------------------- host ----

def _quant_act(x, scale):
    # x [B, L, E] f32; returns int8 [B, E, L] (transposed) and gamma*scale [B, L]
    g = jnp.maximum(jnp.max(jnp.abs(x), axis=-1), EPS)
    qx = jnp.clip(jnp.round(x * (QF / g)[..., None]), -QF, QF - 1.0)
    return jnp.swapaxes(qx, 1, 2).astype(jnp.int8), g * scale


def _act_prep(q, k, v, qs, ks, vs):
    qqT, gq = _quant_act(q, qs / (QF * SQRTD))
    qkT, gk = _quant_act(k, ks / QF)
    qvT, gv = _quant_act(v, vs / QF)
    cores = list(range(NCORES))
    qT_g = jnp.concatenate(
        [qqT[c // 2, :, (c % 2) * LQ:(c % 2 + 1) * LQ] for c in cores], 0)
    kT_g = jnp.concatenate([qkT[c // 2] for c in cores], 0)
    vT_g = jnp.concatenate([qvT[c // 2] for c in cores], 0)
    gq_g = jnp.stack([gq[c // 2, (c % 2) * LQ:(c % 2 + 1) * LQ] for c in cores], 0)
    gk_g = jnp.stack([gk[c // 2] for c in cores], 0)
    gv_g = jnp.concatenate(
        [gv[c // 2].reshape(TK, P).T for c in cores], 0)
    return qT_g, kT_g, vT_g, gq_g, gk_g, gv_g


def _quantize_weight(w):
    s = jnp.maximum(jnp.mean(jnp.abs(w)), EPS)
    qw = jnp.clip(jnp.round(w / s), -1.0, 1.0)
    return qw, s


def _pack_ternary(w):
    # w [E_out, E_in] ternary f32 -> packed int8 [E_in, 256] (base-27 along
    # e_out quarters of the transposed [E_in, E_out] matrix)
    wT = jnp.transpose(w).astype(jnp.int8)
    return (27 * wT[:, 0:256] + 9 * wT[:, 256:512]
            + 3 * wT[:, 512:768] + wT[:, 768:1024]).astype(jnp.int8)


def _weight_prep(ipw, ipb, opw, opb):
    qw_, kw_, vw_ = jnp.split(ipw, 3, 0)
    qb, kb, vb = jnp.split(ipb, 3, 0)
    qqw, qs = _quantize_weight(qw_)
    kqw, ks = _quantize_weight(kw_)
    vqw, vs = _quantize_weight(vw_)
    oqw, os_ = _quantize_weight(opw)
    packs = tuple(_pack_ternary(w) for w in (qqw, kqw, vqw, oqw))
    qbc = (qb / SQRTD).reshape(EC, P).T.astype(jnp.float32)
    kbc = kb.reshape(EC, P).T.astype(jnp.float32)
    vbr = vb[None, :].astype(jnp.float16)
    obr = opb[None, :].astype(jnp.float32)
    return packs, qbc, kbc, vbr, obr, qs, ks, vs, os_


# ---------------------------------------------------------------- runner ----

_CACHE: dict = {}
_WCACHE: dict = {}


def _io_layout(nc):
    in_names, out_names, out_avals = [], [], []
    for alloc in nc.m.functions[0].allocations:
        if not isinstance(alloc, mybir.MemoryLocationSet):
            continue
        name = alloc.memorylocations[0].name
        if alloc.kind == "ExternalInput":
            in_names.append(name)
        elif alloc.kind == "ExternalOutput":
            out_names.append(name)
            out_avals.append(jax.core.ShapedArray(
                tuple(alloc.tensor_shape), mybir.dt.np(alloc.dtype)))
    return in_names, out_names, out_avals


def _get_compiled(os_imm: float):
    key = round(float(os_imm), 12)
    if key in _CACHE:
        return _CACHE[key]
    install_neuronx_cc_hook()
    nc = _build(os_imm)
    in_names, out_names, out_avals = _io_layout(nc)
    part_name = (nc.partition_id_tensor.name
                 if nc.partition_id_tensor else None)
    if part_name is not None:
        in_names = [n for n in in_names if n != part_name]
    n_params = len(in_names)
    all_names = in_names + out_names
    if part_name is not None:
        all_names = all_names + [part_name]
    devices = jax.devices()[:NCORES]
    mesh = Mesh(np.asarray(devices), ("core",))
    sharding = NamedSharding(mesh, PartitionSpec("core"))

    def _body(*args):
        operands = list(args)
        if part_name is not None:
            operands.append(partition_id_tensor())
        outs = _bass_exec_p.bind(
            *operands,
            out_avals=tuple(out_avals),
            in_names=tuple(all_names),
            out_names=tuple(out_names),
            lowering_input_output_aliases=(),
            sim_require_finite=True,
            sim_require_nnan=True,
            nc=nc,
        )
        return tuple(outs)

    donate = tuple(range(n_params, n_params + len(out_names)))
    in_specs = (PartitionSpec("core"),) * (n_params + len(out_names))
    out_specs = (PartitionSpec("core"),) * len(out_names)
    fn = jax.jit(
        shard_map(_body, mesh=mesh, in_specs=in_specs, out_specs=out_specs,
                  check_rep=False),
        donate_argnums=donate, keep_unused=True)

    zinfo = [(tuple(a.shape), a.dtype) for a in out_avals]

    def _mkzeros():
        return tuple(jnp.zeros((NCORES * s[0],) + s[1:], d) for s, d in zinfo)

    zeros_fn = jax.jit(_mkzeros,
                       out_shardings=tuple(sharding for _ in zinfo))
    entry = (fn, zeros_fn, in_names, out_names, sharding)
    _CACHE[key] = entry
    return entry


def _fingerprint(*arrs):
    return tuple((a.shape, str(a.dtype), zlib.adler32(a.tobytes()))
                 for a in arrs)


_CPU = None


def _cpu():
    global _CPU
    if _CPU is None:
        _CPU = jax.devices("cpu")[0]
    return _CPU


def _run(inputs, trace=False, **_ignored):
    cpu = _cpu()
    ipw = np.asarray(inputs["in_proj_weight"], np.float32)
    ipb = np.asarray(inputs["in_proj_bias"], np.float32)
    opw = np.asarray(inputs["out_proj_weight"], np.float32)
    opb = np.asarray(inputs["out_proj_bias"], np.float32)

    wkey = _fingerprint(ipw, ipb, opw, opb)
    if wkey not in _WCACHE:
        with jax.default_device(cpu):
            wjit = jax.jit(_weight_prep)
            packs, qbc, kbc, vbr, obr, qs, ks, vs, os_ = wjit(
                jax.device_put(ipw, cpu), jax.device_put(ipb, cpu),
                jax.device_put(opw, cpu), jax.device_put(opb, cpu))
            packs = [np.asarray(p) for p in packs]
            qbc, kbc = np.asarray(qbc), np.asarray(kbc)
            vbr, obr = np.asarray(vbr), np.asarray(obr)
            qs, ks, vs, os_ = (float(qs), float(ks), float(vs), float(os_))
        fn, zeros_fn, in_names, out_names, sharding = _get_compiled(os_)
        # weight-derived global arrays, committed to device once
        wdev = {
            "wq": jax.device_put(np.tile(packs[0], (NCORES, 1)), sharding),
            "wk": jax.device_put(np.tile(packs[1], (NCORES, 1)), sharding),
            "wv": jax.device_put(np.tile(packs[2], (NCORES, 1)), sharding),
            "wo": jax.device_put(np.tile(packs[3], (NCORES, 1)), sharding),
            "qb": jax.device_put(np.tile(qbc, (NCORES, 1)), sharding),
            "kb": jax.device_put(np.tile(kbc, (NCORES, 1)), sharding),
            "vb": jax.device_put(np.tile(vbr, (NCORES, 1)), sharding),
            "ob": jax.device_put(np.tile(obr, (NCORES, 1)), sharding),
        }
        for a in wdev.values():
            a.block_until_ready()
        _WCACHE[wkey] = (wdev, qs, ks, vs, os_)
    wdev, qs, ks, vs, os_ = _WCACHE[wkey]
    fn, zeros_fn, in_names, out_names, sharding = _get_compiled(os_)

    query = np.asarray(inputs["query"], np.float32)
    key = np.asarray(inputs["key"], np.float32)
    value = np.asarray(inputs["value"], np.float32)
    with jax.default_device(cpu):
        ajit = _CACHE.setdefault("_ajit", jax.jit(_act_prep))
        qT_g, kT_g, vT_g, gq_g, gk_g, gv_g = ajit(
            jax.device_put(query, cpu), jax.device_put(key, cpu),
            jax.device_put(value, cpu),
            jnp.float32(qs), jnp.float32(ks), jnp.float32(vs))
        acts = {
            "qT": np.asarray(qT_g), "kT": np.asarray(kT_g),
            "vT": np.asarray(vT_g), "gq": np.asarray(gq_g),
            "gk": np.asarray(gk_g), "gv": np.asarray(gv_g),
        }

    zeros = zeros_fn()
    args = [wdev[n] if n in wdev else acts[n] for n in in_names]
    outs = fn(*args, *zeros)
    out_map = {n: np.asarray(o) for n, o in zip(out_names, outs)}

    oq = out_map["oq"].reshape(NCORES, LQ, E).astype(np.float32)
    og = out_map["og"].reshape(NCORES, P, TQ)
    out = np.empty((B, L, E), np.float32)
    for c in range(NCORES):
        bb, qh = c // 2, c % 2
        gscale = (og[c].T.reshape(LQ) / OQ).astype(np.float32)
        out[bb, qh * LQ:(qh + 1) * LQ, :] = oq[c] * gscale[:, None]

    class _Res:
        exec_time_ns = None

    return out, _Res()


def kernel(**inputs) -> np.ndarray:
    out, _ = _run(inputs)
    return out


# revision 7
# speedup vs baseline: 1.3426x; 1.3426x over previous
"""BitMultiheadAttention (1.58-bit, inference) on 8 Trainium2 NeuronCores.

The metric for this problem is warm wall-clock of `kernel(**inputs)`, which
is dominated by host<->device transfer over the axon tunnel (~50 MB/s in,
~25 MB/s out).  The design therefore minimizes shipped bytes:

  - activations are quantized to int8 on the host (the reference quantizes
    them to 8 bits anyway: qx = clip(round(x*128/gamma), -128, 127)) and
    shipped pre-transposed [E, tokens] together with tiny per-token scale
    rows; 96 MB fp32 -> 24 MB int8.
  - ternary weights are packed 4-per-byte (base-27: 27*w0+9*w1+3*w2+w3,
    values in [-40, 40]) and unpacked on device with a few DVE ops;
    64 MB fp16 -> 8 MB.  Weight-derived device arrays are cached keyed on
    a fingerprint, so repeat calls with the same weights ship nothing.
  - the output is quantized on device to int8 with a per-token scale
    (error <= rowmax/254, far under the 2e-2 tolerance); 32 MB -> 8 MB.
  - the donated output buffers are created on device (the stock runner
    ships 32 MB of host zeros per call).

Sharding: core c -> batch b = c//2, query-token half = c%2.  key/value of
the batch are replicated to both cores of a pair; no collectives.

Per-core device pipeline (all matmuls fp16 operands, fp32 PSUM):
  1. unpack ternary weights: cast-DMA packed int8 -> fp16, peel base-27
     digits with round(x/b) via the fp16 magic-number trick.
  2. K^T/Q^T computed directly in [e, t] layout: psum[e,t] = Wk @ qx^T,
     dequant = psum * (ws*gamma_t/128) (broadcast row) + bias (per-e col).
     1/sqrt(D) and bias/sqrt(D) are folded into the Q scales on host.
  3. V in [t, e] stride-66 per-head layout (64 data + ones column which
     produces the softmax denominator): psum[t,e] = qx^T.T @ Wv, dequant
     via per-token activation scale + bias row.
  4. attention per head pair exactly as the fp16 flash-style original:
     S^T[k,q] = K^T.T @ Q^T, exp on ACT (scores are O(1), no max needed),
     ctx^T[d,q] accumulated over k-chunks with the ones-row denominator.
  5. softmax normalization, transpose to [t, e], reference-style 8-bit
     re-quantization, out-projection, per-token int8 output quant.
"""

import sys
import zlib
import functools
from contextlib import ExitStack

for _p in ("/opt/trn_rl_repo",):
    if _p not in sys.path:
        sys.path.insert(0, _p)

import numpy as np
import jax
import jax.numpy as jnp

import concourse.bass as bass
import concourse.tile as tile
from concourse import mybir
from concourse.bass2jax import (_bass_exec_p, install_neuronx_cc_hook,
                                partition_id_tensor)
from jax.experimental.shard_map import shard_map
from jax.sharding import Mesh, PartitionSpec, NamedSharding

P = 128
B, L, E, H, D = 4, 2048, 1024, 16, 64
NCORES = 8
LQ = L // 2
EPS = 1e-5
QF = 128.0
MAGIC = 1536.0
SQRTD = 8.0
OQ = 127.0  # output shipping quant range
F32 = mybir.dt.float32
F16 = mybir.dt.float16
I8 = mybir.dt.int8
AX = mybir.AxisListType.X
OP = mybir.AluOpType
EXP = mybir.ActivationFunctionType.Exp
COPY = mybir.ActivationFunctionType.Copy

VSTRIDE = 66
TK = L // P    # 16 k/v token tiles
TQ = LQ // P   # 8 q token tiles
EC = E // P    # 8 chunks of embedding dim


# ---------------------------------------------------------------- device ----

def _emit(ctx: ExitStack, tc: tile.TileContext, io: dict, os_imm: float):
    nc = tc.nc

    res = ctx.enter_context(tc.tile_pool(name="res", bufs=1))
    kT = [res.tile([P, L], F16, tag=f"kT{c}", name=f"kT{c}") for c in range(EC)]
    qT = [res.tile([P, LQ], F16, tag=f"qT{c}", name=f"qT{c}") for c in range(EC)]
    vres = [res.tile([P, H * VSTRIDE], F16, tag=f"v{t}", name=f"v{t}")
            for t in range(TK)]
    ctxT = [res.tile([P, E], F16, tag=f"ctxT{t}", name=f"ctxT{t}")
            for t in range(TQ)]
    # per-e-chunk bias columns for K/Q (f32 [128, EC])
    bcol = res.tile([P, 2 * EC], F32, tag="bcol", name="bcol")
    nc.gpsimd.dma_start(bcol[:, 0:EC], io["kb"][:])
    nc.gpsimd.dma_start(bcol[:, EC:2 * EC], io["qb"][:])
    gvt = res.tile([P, TK], F32, tag="gvt", name="gvt")
    nc.gpsimd.dma_start(gvt[:], io["gv"][:])

    dram = ctx.enter_context(tc.tile_pool(name="dram", bufs=1, space="DRAM"))
    rs_dram = dram.tile([H, LQ], F32, tag="rs", name="rs")
    cn_dram = [dram.tile([64, LQ], F16, tag=f"cnd{h}", name=f"cnd{h}")
               for h in range(H)]
    qn_dram = dram.tile([LQ, E], F16, tag="qnd", name="qnd")

    for t in range(TK):
        ones_ap = vres[t][:].rearrange("p (h c) -> p h c", c=VSTRIDE)[:, :, 64:65]
        nc.vector.memset(ones_ap, 1.0)

    def unpack_weights(stk: ExitStack, name, wdram):
        """Packed base-27 ternary [E, 256] int8 -> 8 fp16 tiles [128, E]."""
        sp = stk.enter_context(tc.tile_pool(name=f"ws_{name}", bufs=2))
        tp = stk.enter_context(tc.tile_pool(name=f"wt_{name}", bufs=4))
        wp = stk.enter_context(tc.tile_pool(name=f"w_{name}", bufs=1))
        w16 = [wp.tile([P, E], F16, tag=f"{name}{i}", name=f"{name}{i}")
               for i in range(EC)]
        for i in range(EC):
            pk = sp.tile([P, 256], F16, tag="pk", name="pk")
            nc.gpsimd.dma_start(pk[:], wdram[i * P:(i + 1) * P, :])
            rem = pk
            for lvl, base in ((0, 27.0), (1, 9.0), (2, 3.0)):
                q = w16[i][:, lvl * 256:(lvl + 1) * 256]
                d = tp.tile([P, 256], F16, tag="d", name="d")
                nc.vector.tensor_scalar(d[:], rem[:], 1.0 / base, MAGIC,
                                        OP.mult, OP.add)
                nc.vector.tensor_scalar(q, d[:], -MAGIC, None, OP.add)
                dst = (tp.tile([P, 256], F16, tag="r", name="r")[:]
                       if lvl < 2 else w16[i][:, 768:1024])
                nc.vector.scalar_tensor_tensor(dst, q, -base, rem[:],
                                               OP.mult, OP.add)
                rem = dst
        return w16

    def kq_phase(stk, name, wdram, xdram, grow_dram, ntok, out_T, bias_off):
        """out_T[e, t] = W @ qx^T, dequant via broadcast gamma row + bias col."""
        w16 = unpack_weights(stk, name, wdram)
        xp = stk.enter_context(tc.tile_pool(name=f"x_{name}", bufs=1))
        x16 = [xp.tile([P, ntok], F16, tag=f"x{i}", name=f"x{i}")
               for i in range(EC)]
        for i in range(EC):
            nc.gpsimd.dma_start(x16[i][:], xdram[i * P:(i + 1) * P, :])
        gb = xp.tile([P, ntok], F32, tag="gb", name="gb")
        nc.gpsimd.dma_start(gb[:], grow_dram[0:1, :].to_broadcast((P, ntok)))
        pp = stk.enter_context(tc.tile_pool(name=f"ps_{name}", bufs=4,
                                            space="PSUM"))
        tp = stk.enter_context(tc.tile_pool(name=f"t_{name}", bufs=4))
        for e in range(EC):
            for ts in range(ntok // 512):
                ps = pp.tile([P, 512], F32, tag="ps", name="ps")
                for i in range(EC):
                    nc.tensor.matmul(ps[:],
                                     lhsT=w16[i][:, e * P:(e + 1) * P],
                                     rhs=x16[i][:, ts * 512:(ts + 1) * 512],
                                     start=(i == 0), stop=(i == EC - 1))
                tmp = tp.tile([P, 512], F32, tag="tmp", name="tmp")
                nc.vector.tensor_tensor(tmp[:], ps[:],
                                        gb[:, ts * 512:(ts + 1) * 512],
                                        op=OP.mult)
                nc.vector.tensor_scalar(out_T[e][:, ts * 512:(ts + 1) * 512],
                                        tmp[:], bcol[:, bias_off + e:bias_off + e + 1],
                                        None, OP.add)

    # --- K^T, Q^T ---
    with ExitStack() as stk:
        kq_phase(stk, "k", io["wk"], io["kT"], io["gk"], L, kT, 0)
    with ExitStack() as stk:
        kq_phase(stk, "q", io["wq"], io["qT"], io["gq"], LQ, qT, EC)

    # --- V (dequant straight into the stride-66 per-head layout) ---
    with ExitStack() as stk:
        wv16 = unpack_weights(stk, "v", io["wv"])
        xp = stk.enter_context(tc.tile_pool(name="x_v", bufs=1))
        vx16 = [xp.tile([P, L], F16, tag=f"vx{i}", name=f"vx{i}")
                for i in range(EC)]
        for i in range(EC):
            nc.gpsimd.dma_start(vx16[i][:], io["vT"][i * P:(i + 1) * P, :])
        vbb = xp.tile([P, E], F16, tag="vbb", name="vbb")
        nc.gpsimd.dma_start(vbb[:], io["vb"][0:1, :].to_broadcast((P, E)))
        pp = stk.enter_context(tc.tile_pool(name="ps_v", bufs=4, space="PSUM"))
        tp = stk.enter_context(tc.tile_pool(name="t_v", bufs=4))
        for tt in range(TK):
            for eh in range(2):
                ps = pp.tile([P, 512], F32, tag="ps", name="ps")
                for i in range(EC):
                    nc.tensor.matmul(ps[:],
                                     lhsT=vx16[i][:, tt * P:(tt + 1) * P],
                                     rhs=wv16[i][:, eh * 512:(eh + 1) * 512],
                                     start=(i == 0), stop=(i == EC - 1))
                tmp = tp.tile([P, 512], F16, tag="tmp", name="tmp")
                nc.scalar.activation(tmp[:], ps[:], COPY,
                                     scale=gvt[:, tt:tt + 1])
                out_ap = (vres[tt][:, eh * 8 * VSTRIDE:(eh * 8 + 8) * VSTRIDE]
                          .rearrange("p (h c) -> p h c", c=VSTRIDE)[:, :, 0:64])
                nc.vector.tensor_tensor(out_ap, tmp[:],
                                        vbb[:, eh * 512:(eh + 1) * 512],
                                        op=OP.add)

    # ---------------- attention ----------------
    with ExitStack() as stk:
        sp = stk.enter_context(tc.tile_pool(name="spsum", bufs=2, space="PSUM"))
        cp = stk.enter_context(tc.tile_pool(name="cpsum", bufs=1, space="PSUM"))
        ptp = stk.enter_context(tc.tile_pool(name="pt", bufs=3))
        c65p = stk.enter_context(tc.tile_pool(name="c65", bufs=4))
        cnp = stk.enter_context(tc.tile_pool(name="cn", bufs=4))
        rsp = stk.enter_context(tc.tile_pool(name="rsbc", bufs=3))

        for hp in range(H // 2):
            ctx_ps = {}
            for hh in range(2):
                for qc in range(2):
                    ctx_ps[(hh, qc)] = cp.tile([65, 512], F32, tag=f"c{hh}{qc}",
                                               name=f"c{hh}{qc}")
            for kc in range(TK):
                for hh in range(2):
                    h = 2 * hp + hh
                    s_ps = sp.tile([P, LQ], F32, tag="s", name="s")
                    for qc in range(2):
                        nc.tensor.matmul(
                            s_ps[:, qc * 512:(qc + 1) * 512],
                            lhsT=kT[hp][hh * 64:(hh + 1) * 64,
                                        kc * P:(kc + 1) * P],
                            rhs=qT[hp][hh * 64:(hh + 1) * 64,
                                       qc * 512:(qc + 1) * 512],
                            start=True, stop=True)
                    pt = ptp.tile([P, LQ], F16, tag="pt", name="pt")
                    nc.scalar.activation(pt[:], s_ps[:], EXP)
                    for qc in range(2):
                        nc.tensor.matmul(
                            ctx_ps[(hh, qc)][:],
                            lhsT=vres[kc][:, h * VSTRIDE:h * VSTRIDE + 65],
                            rhs=pt[:, qc * 512:(qc + 1) * 512],
                            start=(kc == 0), stop=(kc == TK - 1))
            for hh in range(2):
                h = 2 * hp + hh
                c65 = c65p.tile([65, LQ], F32, tag="c65", name="c65")
                for qc in range(2):
                    nc.vector.tensor_copy(c65[:, qc * 512:(qc + 1) * 512],
                                          ctx_ps[(hh, qc)][:])
                nc.vector.reciprocal(c65[64:65, :], c65[64:65, :])
                nc.sync.dma_start(rs_dram[h:h + 1, :], c65[64:65, :])
                rst = rsp.tile([64, LQ], F32, tag="rst", name="rst")
                nc.gpsimd.dma_start(rst[:],
                                    rs_dram[h:h + 1, :].to_broadcast((64, LQ)))
                cn = cnp.tile([64, LQ], F16, tag="cn", name="cn")
                nc.vector.tensor_tensor(cn[:], c65[0:64, :], rst[:], op=OP.mult)
                nc.gpsimd.dma_start(cn_dram[h][:], cn[:])
                for tt in range(TQ):
                    nc.sync.dma_start_transpose(
                        ctxT[tt][:, h * 64:(h + 1) * 64],
                        cn_dram[h][:, tt * P:(tt + 1) * P])

    # ---------------- out-projection ----------------
    with ExitStack() as stk:
        wo16 = unpack_weights(stk, "o", io["wo"])
        smp = stk.enter_context(tc.tile_pool(name="smalls", bufs=6))
        qnp = stk.enter_context(tc.tile_pool(name="qn", bufs=3))
        qcp = stk.enter_context(tc.tile_pool(name="qctx", bufs=1))
        opp = stk.enter_context(tc.tile_pool(name="ops", bufs=4, space="PSUM"))
        outp = stk.enter_context(tc.tile_pool(name="out", bufs=3))
        ogp = stk.enter_context(tc.tile_pool(name="og", bufs=1))

        obb = qcp.tile([P, E], F32, tag="obb", name="obb")
        nc.gpsimd.dma_start(obb[:], io["ob"][0:1, :].to_broadcast((P, E)))
        og_acc = ogp.tile([P, TQ], F32, tag="oga", name="oga")

        qctxT = [qcp.tile([P, LQ], F16, tag=f"qc{c}", name=f"qc{c}")
                 for c in range(EC)]
        d2cols = []
        for tt in range(TQ):
            g = smp.tile([P, 1], F32, tag="g", name="g")
            nc.vector.tensor_reduce(g[:], ctxT[tt][:], axis=AX, op=OP.max,
                                    apply_absolute_value=True)
            nc.vector.tensor_scalar_max(g[:], g[:], EPS)
            s2 = smp.tile([P, 1], F32, tag="s2", name="s2")
            nc.vector.reciprocal(s2[:], g[:])
            nc.vector.tensor_scalar_mul(s2[:], s2[:], QF)
            d2 = smp.tile([P, 1], F32, tag="d2", name="d2")
            nc.vector.tensor_scalar_mul(d2[:], g[:], os_imm / QF)
            d2cols.append(d2)

            qm = qnp.tile([P, E], F16, tag="qm", name="qm")
            nc.vector.tensor_scalar(qm[:], ctxT[tt][:], s2[:], MAGIC,
                                    OP.mult, OP.add)
            qn = qnp.tile([P, E], F16, tag="qnt", name="qnt")
            nc.vector.tensor_scalar(qn[:], qm[:], -MAGIC, QF - 1.0,
                                    OP.add, OP.min)
            nc.gpsimd.dma_start(qn_dram[tt * P:(tt + 1) * P, :], qn[:])
            for c in range(EC):
                nc.sync.dma_start_transpose(
                    qctxT[c][:, tt * P:(tt + 1) * P],
                    qn_dram[tt * P:(tt + 1) * P, c * P:(c + 1) * P])

        for tt in range(TQ):
            ot = outp.tile([P, E], F32, tag="ot", name="ot")
            for e in range(2):
                ps = opp.tile([P, 512], F32, tag="ops", name="ops")
                for c in range(EC):
                    nc.tensor.matmul(ps[:],
                                     lhsT=qctxT[c][:, tt * P:(tt + 1) * P],
                                     rhs=wo16[c][:, e * 512:(e + 1) * 512],
                                     start=(c == 0), stop=(c == EC - 1))
                sl = ot[:, e * 512:(e + 1) * 512]
                nc.scalar.activation(sl, ps[:], COPY, scale=d2cols[tt][:])
                nc.vector.tensor_tensor(sl, sl,
                                        obb[:, e * 512:(e + 1) * 512],
                                        op=OP.add)
            # int8 shipping quant: per-token scale = rowmax/127
            go = smp.tile([P, 1], F32, tag="go", name="go")
            nc.vector.tensor_reduce(go[:], ot[:], axis=AX, op=OP.max,
                                    apply_absolute_value=True)
            nc.vector.tensor_scalar_max(go[:], go[:], 1e-30)
            nc.vector.tensor_copy(og_acc[:, tt:tt + 1], go[:])
            ro = smp.tile([P, 1], F32, tag="ro", name="ro")
            nc.vector.reciprocal(ro[:], go[:])
            nc.vector.tensor_scalar_mul(ro[:], ro[:], OQ)
            o16 = outp.tile([P, E], F16, tag="o16", name="o16")
            nc.vector.tensor_scalar(o16[:], ot[:], ro[:], MAGIC,
                                    OP.mult, OP.add)
            o16b = outp.tile([P, E], F16, tag="o16b", name="o16b")
            nc.vector.tensor_scalar(o16b[:], o16[:], -MAGIC, None, OP.add)
            nc.gpsimd.dma_start(io["oq"][tt * P:(tt + 1) * P, :], o16b[:])
        nc.sync.dma_start(io["og"][:], og_acc[:])


def _hoist_excess_waits(nc: bass.Bass):
    """Walrus encodes at most 1 semaphore wait on a DMA DIRECT2D / NoOp and 2
    on compute instruction structs.  Hoist excess waits onto NoOp instructions
    inserted just before the offender on the same engine."""
    import bass_rust
    nwh = 0
    for blk in nc.m.functions[0].blocks:
        insts = blk.instructions
        i = 0
        while i < len(insts):
            ins = insts[i]
            si = ins.sync_info
            limit = 1
            if si is not None and si.on_wait and len(si.on_wait) > limit:
                ow = list(si.on_wait)
                ins.sync_info = bass_rust.SyncInfo(
                    on_wait=[], on_update=list(si.on_update))
                pos = i
                for j in range(len(ow)):
                    nop = mybir.InstNoOp(name=f"WH{nwh}-{ins.name}",
                                         ins=[], outs=[])
                    nop.engine = ins.engine
                    nop.sync_info = bass_rust.SyncInfo(
                        on_wait=[ow[j]], on_update=[])
                    insts.insert(pos, nop)
                    pos += 1
                    nwh += 1
                i = pos + 1
            else:
                i += 1
    return nwh


def _build(os_imm: float) -> bass.Bass:
    nc = bass.Bass(trn_type="TRN2", num_swdge_queues=4)
    io = {
        "qT": nc.dram_tensor("qT", [E, LQ], I8, kind="ExternalInput"),
        "kT": nc.dram_tensor("kT", [E, L], I8, kind="ExternalInput"),
        "vT": nc.dram_tensor("vT", [E, L], I8, kind="ExternalInput"),
        "wq": nc.dram_tensor("wq", [E, 256], I8, kind="ExternalInput"),
        "wk": nc.dram_tensor("wk", [E, 256], I8, kind="ExternalInput"),
        "wv": nc.dram_tensor("wv", [E, 256], I8, kind="ExternalInput"),
        "wo": nc.dram_tensor("wo", [E, 256], I8, kind="ExternalInput"),
        "gq": nc.dram_tensor("gq", [1, LQ], F32, kind="ExternalInput"),
        "gk": nc.dram_tensor("gk", [1, L], F32, kind="ExternalInput"),
        "gv": nc.dram_tensor("gv", [P, TK], F32, kind="ExternalInput"),
        "qb": nc.dram_tensor("qb", [P, EC], F32, kind="ExternalInput"),
        "kb": nc.dram_tensor("kb", [P, EC], F32, kind="ExternalInput"),
        "vb": nc.dram_tensor("vb", [1, E], F16, kind="ExternalInput"),
        "ob": nc.dram_tensor("ob", [1, E], F32, kind="ExternalInput"),
        "oq": nc.dram_tensor("oq", [LQ, E], I8, kind="ExternalOutput"),
        "og": nc.dram_tensor("og", [P, TQ], F32, kind="ExternalOutput"),
    }
    io = {k: v[:] for k, v in io.items()}
    with ExitStack() as ctx:
        tc = ctx.enter_context(tile.TileContext(nc))
        _emit(ctx, tc, io, os_imm)
    _hoist_excess_waits(nc)
    nc.finalize()
    return nc


# ---------------------------------------------------------------- host ----

def _quant_act(x, scale):
    # x [B, L, E] f32; returns int8 [B, E, L] (transposed) and gamma*scale [B, L]
    g = jnp.maximum(jnp.max(jnp.abs(x), axis=-1), EPS)
    qx = jnp.clip(jnp.round(x * (QF / g)[..., None]), -QF, QF - 1.0)
    return jnp.swapaxes(qx, 1, 2).astype(jnp.int8), g * scale


def _q_prep(q, qs):
    qqT, gq = _quant_act(q, qs / (QF * SQRTD))
    cores = list(range(NCORES))
    qT_g = jnp.concatenate(
        [qqT[c // 2, :, (c % 2) * LQ:(c % 2 + 1) * LQ] for c in cores], 0)
    gq_g = jnp.stack([gq[c // 2, (c % 2) * LQ:(c % 2 + 1) * LQ] for c in cores], 0)
    return qT_g, gq_g


def _k_prep(k, ks):
    qkT, gk = _quant_act(k, ks / QF)
    kT_u = qkT.reshape(B * E, L)  # unique data: one copy per batch
    gk_g = jnp.stack([gk[c // 2] for c in range(NCORES)], 0)
    return kT_u, gk_g


def _v_prep(v, vs):
    qvT, gv = _quant_act(v, vs / QF)
    vT_u = qvT.reshape(B * E, L)
    gv_g = jnp.concatenate(
        [gv[c // 2].reshape(TK, P).T for c in range(NCORES)], 0)
    return vT_u, gv_g


def _quantize_weight(w):
    s = jnp.maximum(jnp.mean(jnp.abs(w)), EPS)
    qw = jnp.clip(jnp.round(w / s), -1.0, 1.0)
    return qw, s


def _pack_ternary(w):
    # w [E_out, E_in] ternary f32 -> packed int8 [E_in, 256] (base-27 along
    # e_out quarters of the transposed [E_in, E_out] matrix)
    wT = jnp.transpose(w).astype(jnp.int8)
    return (27 * wT[:, 0:256] + 9 * wT[:, 256:512]
            + 3 * wT[:, 512:768] + wT[:, 768:1024]).astype(jnp.int8)


def _weight_prep(ipw, ipb, opw, opb):
    qw_, kw_, vw_ = jnp.split(ipw, 3, 0)
    qb, kb, vb = jnp.split(ipb, 3, 0)
    qqw, qs = _quantize_weight(qw_)
    kqw, ks = _quantize_weight(kw_)
    vqw, vs = _quantize_weight(vw_)
    oqw, os_ = _quantize_weight(opw)
    packs = tuple(_pack_ternary(w) for w in (qqw, kqw, vqw, oqw))
    qbc = (qb / SQRTD).reshape(EC, P).T.astype(jnp.float32)
    kbc = kb.reshape(EC, P).T.astype(jnp.float32)
    vbr = vb[None, :].astype(jnp.float16)
    obr = opb[None, :].astype(jnp.float32)
    return packs, qbc, kbc, vbr, obr, qs, ks, vs, os_


# ---------------------------------------------------------------- runner ----

_CACHE: dict = {}
_WCACHE: dict = {}


def _io_layout(nc):
    in_names, out_names, out_avals = [], [], []
    for alloc in nc.m.functions[0].allocations:
        if not isinstance(alloc, mybir.MemoryLocationSet):
            continue
        name = alloc.memorylocations[0].name
        if alloc.kind == "ExternalInput":
            in_names.append(name)
        elif alloc.kind == "ExternalOutput":
            out_names.append(name)
            out_avals.append(jax.core.ShapedArray(
                tuple(alloc.tensor_shape), mybir.dt.np(alloc.dtype)))
    return in_names, out_names, out_avals


def _get_compiled(os_imm: float):
    key = round(float(os_imm), 12)
    if key in _CACHE:
        return _CACHE[key]
    install_neuronx_cc_hook()
    nc = _build(os_imm)
    in_names, out_names, out_avals = _io_layout(nc)
    part_name = (nc.partition_id_tensor.name
                 if nc.partition_id_tensor else None)
    if part_name is not None:
        in_names = [n for n in in_names if n != part_name]
    n_params = len(in_names)
    all_names = in_names + out_names
    if part_name is not None:
        all_names = all_names + [part_name]
    devices = jax.devices()[:NCORES]
    mesh = Mesh(np.asarray(devices).reshape(B, 2), ("pair", "half"))
    spec_core = PartitionSpec(("pair", "half"))
    spec_pair = PartitionSpec("pair")
    sh_core = NamedSharding(mesh, spec_core)
    sh_pair = NamedSharding(mesh, spec_pair)
    sh_half = sh_core  # kv halves shipped 8-way, resharded on device

    def _body(*args):
        operands = list(args)
        if part_name is not None:
            operands.append(partition_id_tensor())
        outs = _bass_exec_p.bind(
            *operands,
            out_avals=tuple(out_avals),
            in_names=tuple(all_names),
            out_names=tuple(out_names),
            lowering_input_output_aliases=(),
            sim_require_finite=True,
            sim_require_nnan=True,
            nc=nc,
        )
        return tuple(outs)

    donate = tuple(range(n_params, n_params + len(out_names)))
    in_specs = tuple(
        spec_pair if n in ("kT", "vT") else spec_core for n in in_names
    ) + (spec_core,) * len(out_names)
    out_specs = (spec_core,) * len(out_names)
    fn = jax.jit(
        shard_map(_body, mesh=mesh, in_specs=in_specs, out_specs=out_specs,
                  check_rep=False),
        donate_argnums=donate, keep_unused=True)

    zinfo = [(tuple(a.shape), a.dtype) for a in out_avals]

    def _mkzeros():
        return tuple(jnp.zeros((NCORES * s[0],) + s[1:], d) for s, d in zinfo)

    zeros_fn = jax.jit(_mkzeros,
                       out_shardings=tuple(sh_core for _ in zinfo))
    reshard_fn = jax.jit(lambda a, b: (a, b),
                         out_shardings=(sh_pair, sh_pair))
    entry = (fn, zeros_fn, reshard_fn, in_names, out_names,
             sh_core, sh_half)
    _CACHE[key] = entry
    return entry


def _fingerprint(*arrs):
    return tuple((a.shape, str(a.dtype), zlib.adler32(a.tobytes()))
                 for a in arrs)


_CPU = None


def _cpu():
    global _CPU
    if _CPU is None:
        _CPU = jax.devices("cpu")[0]
    return _CPU


def _run(inputs, trace=False, **_ignored):
    cpu = _cpu()
    ipw = np.asarray(inputs["in_proj_weight"], np.float32)
    ipb = np.asarray(inputs["in_proj_bias"], np.float32)
    opw = np.asarray(inputs["out_proj_weight"], np.float32)
    opb = np.asarray(inputs["out_proj_bias"], np.float32)

    wkey = _fingerprint(ipw, ipb, opw, opb)
    if wkey not in _WCACHE:
        with jax.default_device(cpu):
            wjit = jax.jit(_weight_prep)
            packs, qbc, kbc, vbr, obr, qs, ks, vs, os_ = wjit(
                jax.device_put(ipw, cpu), jax.device_put(ipb, cpu),
                jax.device_put(opw, cpu), jax.device_put(opb, cpu))
            packs = [np.asarray(p) for p in packs]
            qbc, kbc = np.asarray(qbc), np.asarray(kbc)
            vbr, obr = np.asarray(vbr), np.asarray(obr)
            qs, ks, vs, os_ = (float(qs), float(ks), float(vs), float(os_))
        (fn, zeros_fn, reshard_fn, in_names, out_names,
         sh_core, sh_half) = _get_compiled(os_)
        # weight-derived global arrays, committed to device once
        wdev = {
            "wq": jax.device_put(np.tile(packs[0], (NCORES, 1)), sh_core),
            "wk": jax.device_put(np.tile(packs[1], (NCORES, 1)), sh_core),
            "wv": jax.device_put(np.tile(packs[2], (NCORES, 1)), sh_core),
            "wo": jax.device_put(np.tile(packs[3], (NCORES, 1)), sh_core),
            "qb": jax.device_put(np.tile(qbc, (NCORES, 1)), sh_core),
            "kb": jax.device_put(np.tile(kbc, (NCORES, 1)), sh_core),
            "vb": jax.device_put(np.tile(vbr, (NCORES, 1)), sh_core),
            "ob": jax.device_put(np.tile(obr, (NCORES, 1)), sh_core),
        }
        for a in wdev.values():
            a.block_until_ready()
        _WCACHE[wkey] = (wdev, qs, ks, vs, os_)
    wdev, qs, ks, vs, os_ = _WCACHE[wkey]
    (fn, zeros_fn, reshard_fn, in_names, out_names,
     sh_core, sh_half) = _get_compiled(os_)

    zeros = zeros_fn()  # on-device, async
    query = np.asarray(inputs["query"], np.float32)
    key = np.asarray(inputs["key"], np.float32)
    value = np.asarray(inputs["value"], np.float32)
    # pipeline: per-tensor cpu quant -> async shard transfer; kv ship as
    # 8-way halves (unique bytes only) and get pair-replicated on device.
    with jax.default_device(cpu):
        kjit = _CACHE.setdefault("_kjit", jax.jit(_k_prep))
        vjit = _CACHE.setdefault("_vjit", jax.jit(_v_prep))
        qjit = _CACHE.setdefault("_qjit", jax.jit(_q_prep))
        kT_u, gk_g = kjit(key, jnp.float32(ks))
        kh_d = jax.device_put(kT_u, sh_half)
        vT_u, gv_g = vjit(value, jnp.float32(vs))
        vh_d = jax.device_put(vT_u, sh_half)
        qT_g, gq_g = qjit(query, jnp.float32(qs))
        qT_d = jax.device_put(qT_g, sh_core)
    kf_d, vf_d = reshard_fn(kh_d, vh_d)
    acts = {
        "qT": qT_d, "kT": kf_d, "vT": vf_d,
        "gq": np.asarray(gq_g), "gk": np.asarray(gk_g),
        "gv": np.asarray(gv_g),
    }
    args = [wdev[n] if n in wdev else acts[n] for n in in_names]
    outs = fn(*args, *zeros)
    out_map = {n: np.asarray(o) for n, o in zip(out_names, outs)}

    oq = out_map["oq"].reshape(NCORES, LQ, E).astype(np.float32)
    og = out_map["og"].reshape(NCORES, P, TQ)
    out = np.empty((B, L, E), np.float32)
    for c in range(NCORES):
        bb, qh = c // 2, c % 2
        gscale = (og[c].T.reshape(LQ) / OQ).astype(np.float32)
        out[bb, qh * LQ:(qh + 1) * LQ, :] = oq[c] * gscale[:, None]

    class _Res:
        exec_time_ns = None

    return out, _Res()


def kernel(**inputs) -> np.ndarray:
    out, _ = _run(inputs)
    return out


# revision 15
# speedup vs baseline: 1.3956x; 1.0395x over previous
"""BitMultiheadAttention (1.58-bit, inference) on 8 Trainium2 NeuronCores.

The metric for this problem is warm wall-clock of `kernel(**inputs)`, which
is dominated by host<->device transfer over the axon tunnel (~50 MB/s in,
~25 MB/s out).  The design therefore minimizes shipped bytes:

  - activations are quantized to int8 on the host (the reference quantizes
    them to 8 bits anyway: qx = clip(round(x*128/gamma), -128, 127)) and
    shipped pre-transposed [E, tokens] together with tiny per-token scale
    rows; 96 MB fp32 -> 24 MB int8.
  - ternary weights are packed 4-per-byte (base-27: 27*w0+9*w1+3*w2+w3,
    values in [-40, 40]) and unpacked on device with a few DVE ops;
    64 MB fp16 -> 8 MB.  Weight-derived device arrays are cached keyed on
    a fingerprint, so repeat calls with the same weights ship nothing.
  - the output is quantized on device to int8 with a per-token scale
    (error <= rowmax/254, far under the 2e-2 tolerance); 32 MB -> 8 MB.
  - the donated output buffers are created on device (the stock runner
    ships 32 MB of host zeros per call).

Sharding: core c -> batch b = c//2, query-token half = c%2.  key/value of
the batch are replicated to both cores of a pair; no collectives.

Per-core device pipeline (all matmuls fp16 operands, fp32 PSUM):
  1. unpack ternary weights: cast-DMA packed int8 -> fp16, peel base-27
     digits with round(x/b) via the fp16 magic-number trick.
  2. K^T/Q^T computed directly in [e, t] layout: psum[e,t] = Wk @ qx^T,
     dequant = psum * (ws*gamma_t/128) (broadcast row) + bias (per-e col).
     1/sqrt(D) and bias/sqrt(D) are folded into the Q scales on host.
  3. V in [t, e] stride-66 per-head layout (64 data + ones column which
     produces the softmax denominator): psum[t,e] = qx^T.T @ Wv, dequant
     via per-token activation scale + bias row.
  4. attention per head pair exactly as the fp16 flash-style original:
     S^T[k,q] = K^T.T @ Q^T, exp on ACT (scores are O(1), no max needed),
     ctx^T[d,q] accumulated over k-chunks with the ones-row denominator.
  5. softmax normalization, transpose to [t, e], reference-style 8-bit
     re-quantization, out-projection, per-token int8 output quant.
"""

import sys
import zlib
import functools
from contextlib import ExitStack

for _p in ("/opt/trn_rl_repo",):
    if _p not in sys.path:
        sys.path.insert(0, _p)

import numpy as np
import jax
import jax.numpy as jnp

import concourse.bass as bass
import concourse.tile as tile
from concourse import mybir
from concourse.bass2jax import (_bass_exec_p, install_neuronx_cc_hook,
                                partition_id_tensor)
from jax.experimental.shard_map import shard_map
from jax.sharding import Mesh, PartitionSpec, NamedSharding

P = 128
B, L, E, H, D = 4, 2048, 1024, 16, 64
NCORES = 8
LQ = L // 2
EPS = 1e-5
QF = 128.0
MAGIC = 1536.0
SQRTD = 8.0
OQ = 127.0  # output shipping quant range
F32 = mybir.dt.float32
F16 = mybir.dt.float16
I8 = mybir.dt.int8
AX = mybir.AxisListType.X
OP = mybir.AluOpType
EXP = mybir.ActivationFunctionType.Exp
COPY = mybir.ActivationFunctionType.Copy

VSTRIDE = 66
TK = L // P    # 16 k/v token tiles
TQ = LQ // P   # 8 q token tiles
EC = E // P    # 8 chunks of embedding dim


# ---------------------------------------------------------------- device ----

def _emit(ctx: ExitStack, tc: tile.TileContext, io: dict, os_imm: float):
    nc = tc.nc

    res = ctx.enter_context(tc.tile_pool(name="res", bufs=1))
    kT = [res.tile([P, L], F16, tag=f"kT{c}", name=f"kT{c}") for c in range(EC)]
    qT = [res.tile([P, LQ], F16, tag=f"qT{c}", name=f"qT{c}") for c in range(EC)]
    vres = [res.tile([P, H * VSTRIDE], F16, tag=f"v{t}", name=f"v{t}")
            for t in range(TK)]
    ctxT = [res.tile([P, E], F16, tag=f"ctxT{t}", name=f"ctxT{t}")
            for t in range(TQ)]
    # per-e-chunk bias columns for K/Q (f32 [128, EC])
    bcol = res.tile([P, 2 * EC], F32, tag="bcol", name="bcol")
    nc.gpsimd.dma_start(bcol[:, 0:EC], io["kb"][:])
    nc.gpsimd.dma_start(bcol[:, EC:2 * EC], io["qb"][:])
    gvt = res.tile([P, TK], F32, tag="gvt", name="gvt")
    nc.gpsimd.dma_start(gvt[:], io["gv"][:])

    dram = ctx.enter_context(tc.tile_pool(name="dram", bufs=1, space="DRAM"))
    rs_dram = dram.tile([H, LQ], F32, tag="rs", name="rs")
    cn_dram = [dram.tile([64, LQ], F16, tag=f"cnd{h}", name=f"cnd{h}")
               for h in range(H)]
    qn_dram = dram.tile([LQ, E], F16, tag="qnd", name="qnd")

    # pair-exchange of the kT/vT halves: each core ships E/2 rows; a 2-core
    # AllGather (bounce-buffered — collectives can't touch I/O tensors)
    # reassembles the full [E, L] int8 kT/vT on both cores of the pair.
    kh_b = dram.tile([E // 2, L], I8, tag="khb", name="khb")
    vh_b = dram.tile([E // 2, L], I8, tag="vhb", name="vhb")
    kf = dram.tile([E, L], I8, tag="kf", name="kf")
    vf = dram.tile([E, L], I8, tag="vf", name="vf")
    nc.gpsimd.dma_start(kh_b[:], io["kh"])
    nc.gpsimd.dma_start(vh_b[:], io["vh"])
    pairs = [[2 * i, 2 * i + 1] for i in range(B)]
    nc.gpsimd.collective_compute("AllGather", OP.bypass, pairs,
                                 ins=[kh_b.opt()], outs=[kf.opt()])
    nc.gpsimd.collective_compute("AllGather", OP.bypass, pairs,
                                 ins=[vh_b.opt()], outs=[vf.opt()])
    io = dict(io, kT=kf[:], vT=vf[:])

    for t in range(TK):
        ones_ap = vres[t][:].rearrange("p (h c) -> p h c", c=VSTRIDE)[:, :, 64:65]
        nc.vector.memset(ones_ap, 1.0)

    def unpack_weights(stk: ExitStack, name, wdram):
        """Packed base-27 ternary [E, 256] int8 -> 8 fp16 tiles [128, E]."""
        sp = stk.enter_context(tc.tile_pool(name=f"ws_{name}", bufs=2))
        tp = stk.enter_context(tc.tile_pool(name=f"wt_{name}", bufs=4))
        wp = stk.enter_context(tc.tile_pool(name=f"w_{name}", bufs=1))
        w16 = [wp.tile([P, E], F16, tag=f"{name}{i}", name=f"{name}{i}")
               for i in range(EC)]
        for i in range(EC):
            pk = sp.tile([P, 256], F16, tag="pk", name="pk")
            nc.gpsimd.dma_start(pk[:], wdram[i * P:(i + 1) * P, :])
            rem = pk
            for lvl, base in ((0, 27.0), (1, 9.0), (2, 3.0)):
                q = w16[i][:, lvl * 256:(lvl + 1) * 256]
                d = tp.tile([P, 256], F16, tag="d", name="d")
                nc.vector.tensor_scalar(d[:], rem[:], 1.0 / base, MAGIC,
                                        OP.mult, OP.add)
                nc.vector.tensor_scalar(q, d[:], -MAGIC, None, OP.add)
                dst = (tp.tile([P, 256], F16, tag="r", name="r")[:]
                       if lvl < 2 else w16[i][:, 768:1024])
                nc.vector.scalar_tensor_tensor(dst, q, -base, rem[:],
                                               OP.mult, OP.add)
                rem = dst
        return w16

    def kq_phase(stk, name, wdram, xdram, grow_dram, ntok, out_T, bias_off):
        """out_T[e, t] = W @ qx^T, dequant via broadcast gamma row + bias col."""
        w16 = unpack_weights(stk, name, wdram)
        xp = stk.enter_context(tc.tile_pool(name=f"x_{name}", bufs=1))
        x16 = [xp.tile([P, ntok], F16, tag=f"x{i}", name=f"x{i}")
               for i in range(EC)]
        for i in range(EC):
            nc.gpsimd.dma_start(x16[i][:], xdram[i * P:(i + 1) * P, :])
        gb = xp.tile([P, ntok], F32, tag="gb", name="gb")
        nc.gpsimd.dma_start(gb[:], grow_dram[0:1, :].to_broadcast((P, ntok)))
        pp = stk.enter_context(tc.tile_pool(name=f"ps_{name}", bufs=4,
                                            space="PSUM"))
        tp = stk.enter_context(tc.tile_pool(name=f"t_{name}", bufs=4))
        for e in range(EC):
            for ts in range(ntok // 512):
                ps = pp.tile([P, 512], F32, tag="ps", name="ps")
                for i in range(EC):
                    nc.tensor.matmul(ps[:],
                                     lhsT=w16[i][:, e * P:(e + 1) * P],
                                     rhs=x16[i][:, ts * 512:(ts + 1) * 512],
                                     start=(i == 0), stop=(i == EC - 1))
                tmp = tp.tile([P, 512], F32, tag="tmp", name="tmp")
                nc.vector.tensor_tensor(tmp[:], ps[:],
                                        gb[:, ts * 512:(ts + 1) * 512],
                                        op=OP.mult)
                nc.vector.tensor_scalar(out_T[e][:, ts * 512:(ts + 1) * 512],
                                        tmp[:], bcol[:, bias_off + e:bias_off + e + 1],
                                        None, OP.add)

    # --- K^T, Q^T ---
    with ExitStack() as stk:
        kq_phase(stk, "k", io["wk"], io["kT"], io["gk"], L, kT, 0)
    with ExitStack() as stk:
        kq_phase(stk, "q", io["wq"], io["qT"], io["gq"], LQ, qT, EC)

    # --- V (dequant straight into the stride-66 per-head layout) ---
    with ExitStack() as stk:
        wv16 = unpack_weights(stk, "v", io["wv"])
        xp = stk.enter_context(tc.tile_pool(name="x_v", bufs=1))
        vx16 = [xp.tile([P, L], F16, tag=f"vx{i}", name=f"vx{i}")
                for i in range(EC)]
        for i in range(EC):
            nc.gpsimd.dma_start(vx16[i][:], io["vT"][i * P:(i + 1) * P, :])
        vbb = xp.tile([P, E], F16, tag="vbb", name="vbb")
        nc.gpsimd.dma_start(vbb[:], io["vb"][0:1, :].to_broadcast((P, E)))
        pp = stk.enter_context(tc.tile_pool(name="ps_v", bufs=4, space="PSUM"))
        tp = stk.enter_context(tc.tile_pool(name="t_v", bufs=4))
        for tt in range(TK):
            for eh in range(2):
                ps = pp.tile([P, 512], F32, tag="ps", name="ps")
                for i in range(EC):
                    nc.tensor.matmul(ps[:],
                                     lhsT=vx16[i][:, tt * P:(tt + 1) * P],
                                     rhs=wv16[i][:, eh * 512:(eh + 1) * 512],
                                     start=(i == 0), stop=(i == EC - 1))
                tmp = tp.tile([P, 512], F16, tag="tmp", name="tmp")
                nc.scalar.activation(tmp[:], ps[:], COPY,
                                     scale=gvt[:, tt:tt + 1])
                out_ap = (vres[tt][:, eh * 8 * VSTRIDE:(eh * 8 + 8) * VSTRIDE]
                          .rearrange("p (h c) -> p h c", c=VSTRIDE)[:, :, 0:64])
                nc.vector.tensor_tensor(out_ap, tmp[:],
                                        vbb[:, eh * 512:(eh + 1) * 512],
                                        op=OP.add)

    # ---------------- attention ----------------
    with ExitStack() as stk:
        sp = stk.enter_context(tc.tile_pool(name="spsum", bufs=2, space="PSUM"))
        cp = stk.enter_context(tc.tile_pool(name="cpsum", bufs=1, space="PSUM"))
        ptp = stk.enter_context(tc.tile_pool(name="pt", bufs=3))
        c65p = stk.enter_context(tc.tile_pool(name="c65", bufs=4))
        cnp = stk.enter_context(tc.tile_pool(name="cn", bufs=4))
        rsp = stk.enter_context(tc.tile_pool(name="rsbc", bufs=3))

        for hp in range(H // 2):
            ctx_ps = {}
            for hh in range(2):
                for qc in range(2):
                    ctx_ps[(hh, qc)] = cp.tile([65, 512], F32, tag=f"c{hh}{qc}",
                                               name=f"c{hh}{qc}")
            for kc in range(TK):
                for hh in range(2):
                    h = 2 * hp + hh
                    s_ps = sp.tile([P, LQ], F32, tag="s", name="s")
                    for qc in range(2):
                        nc.tensor.matmul(
                            s_ps[:, qc * 512:(qc + 1) * 512],
                            lhsT=kT[hp][hh * 64:(hh + 1) * 64,
                                        kc * P:(kc + 1) * P],
                            rhs=qT[hp][hh * 64:(hh + 1) * 64,
                                       qc * 512:(qc + 1) * 512],
                            start=True, stop=True)
                    pt = ptp.tile([P, LQ], F16, tag="pt", name="pt")
                    nc.scalar.activation(pt[:], s_ps[:], EXP)
                    for qc in range(2):
                        nc.tensor.matmul(
                            ctx_ps[(hh, qc)][:],
                            lhsT=vres[kc][:, h * VSTRIDE:h * VSTRIDE + 65],
                            rhs=pt[:, qc * 512:(qc + 1) * 512],
                            start=(kc == 0), stop=(kc == TK - 1))
            for hh in range(2):
                h = 2 * hp + hh
                c65 = c65p.tile([65, LQ], F32, tag="c65", name="c65")
                for qc in range(2):
                    nc.vector.tensor_copy(c65[:, qc * 512:(qc + 1) * 512],
                                          ctx_ps[(hh, qc)][:])
                nc.vector.reciprocal(c65[64:65, :], c65[64:65, :])
                nc.sync.dma_start(rs_dram[h:h + 1, :], c65[64:65, :])
                rst = rsp.tile([64, LQ], F32, tag="rst", name="rst")
                nc.gpsimd.dma_start(rst[:],
                                    rs_dram[h:h + 1, :].to_broadcast((64, LQ)))
                cn = cnp.tile([64, LQ], F16, tag="cn", name="cn")
                nc.vector.tensor_tensor(cn[:], c65[0:64, :], rst[:], op=OP.mult)
                nc.gpsimd.dma_start(cn_dram[h][:], cn[:])
                for tt in range(TQ):
                    nc.sync.dma_start_transpose(
                        ctxT[tt][:, h * 64:(h + 1) * 64],
                        cn_dram[h][:, tt * P:(tt + 1) * P])

    # ---------------- out-projection ----------------
    with ExitStack() as stk:
        wo16 = unpack_weights(stk, "o", io["wo"])
        smp = stk.enter_context(tc.tile_pool(name="smalls", bufs=6))
        qnp = stk.enter_context(tc.tile_pool(name="qn", bufs=3))
        qcp = stk.enter_context(tc.tile_pool(name="qctx", bufs=1))
        opp = stk.enter_context(tc.tile_pool(name="ops", bufs=4, space="PSUM"))
        outp = stk.enter_context(tc.tile_pool(name="out", bufs=3))
        ogp = stk.enter_context(tc.tile_pool(name="og", bufs=1))

        obb = qcp.tile([P, E], F32, tag="obb", name="obb")
        nc.gpsimd.dma_start(obb[:], io["ob"][0:1, :].to_broadcast((P, E)))
        og_acc = ogp.tile([P, TQ], F32, tag="oga", name="oga")

        qctxT = [qcp.tile([P, LQ], F16, tag=f"qc{c}", name=f"qc{c}")
                 for c in range(EC)]
        d2cols = []
        for tt in range(TQ):
            g = smp.tile([P, 1], F32, tag="g", name="g")
            nc.vector.tensor_reduce(g[:], ctxT[tt][:], axis=AX, op=OP.max,
                                    apply_absolute_value=True)
            nc.vector.tensor_scalar_max(g[:], g[:], EPS)
            s2 = smp.tile([P, 1], F32, tag="s2", name="s2")
            nc.vector.reciprocal(s2[:], g[:])
            nc.vector.tensor_scalar_mul(s2[:], s2[:], QF)
            d2 = smp.tile([P, 1], F32, tag="d2", name="d2")
            nc.vector.tensor_scalar_mul(d2[:], g[:], os_imm / QF)
            d2cols.append(d2)

            qm = qnp.tile([P, E], F16, tag="qm", name="qm")
            nc.vector.tensor_scalar(qm[:], ctxT[tt][:], s2[:], MAGIC,
                                    OP.mult, OP.add)
            qn = qnp.tile([P, E], F16, tag="qnt", name="qnt")
            nc.vector.tensor_scalar(qn[:], qm[:], -MAGIC, QF - 1.0,
                                    OP.add, OP.min)
            nc.gpsimd.dma_start(qn_dram[tt * P:(tt + 1) * P, :], qn[:])
            for c in range(EC):
                nc.sync.dma_start_transpose(
                    qctxT[c][:, tt * P:(tt + 1) * P],
                    qn_dram[tt * P:(tt + 1) * P, c * P:(c + 1) * P])

        for tt in range(TQ):
            ot = outp.tile([P, E], F32, tag="ot", name="ot")
            for e in range(2):
                ps = opp.tile([P, 512], F32, tag="ops", name="ops")
                for c in range(EC):
                    nc.tensor.matmul(ps[:],
                                     lhsT=qctxT[c][:, tt * P:(tt + 1) * P],
                                     rhs=wo16[c][:, e * 512:(e + 1) * 512],
                                     start=(c == 0), stop=(c == EC - 1))
                sl = ot[:, e * 512:(e + 1) * 512]
                nc.scalar.activation(sl, ps[:], COPY, scale=d2cols[tt][:])
                nc.vector.tensor_tensor(sl, sl,
                                        obb[:, e * 512:(e + 1) * 512],
                                        op=OP.add)
            # int8 shipping quant: per-token scale = rowmax/127
            go = smp.tile([P, 1], F32, tag="go", name="go")
            nc.vector.tensor_reduce(go[:], ot[:], axis=AX, op=OP.max,
                                    apply_absolute_value=True)
            nc.vector.tensor_scalar_max(go[:], go[:], 1e-30)
            nc.vector.tensor_copy(og_acc[:, tt:tt + 1], go[:])
            ro = smp.tile([P, 1], F32, tag="ro", name="ro")
            nc.vector.reciprocal(ro[:], go[:])
            nc.vector.tensor_scalar_mul(ro[:], ro[:], OQ)
            o16 = outp.tile([P, E], F16, tag="o16", name="o16")
            nc.vector.tensor_scalar(o16[:], ot[:], ro[:], MAGIC,
                                    OP.mult, OP.add)
            o16b = outp.tile([P, E], F16, tag="o16b", name="o16b")
            nc.vector.tensor_scalar(o16b[:], o16[:], -MAGIC, None, OP.add)
            nc.gpsimd.dma_start(io["oq"][tt * P:(tt + 1) * P, :], o16b[:])
        nc.sync.dma_start(io["og"][:], og_acc[:])


def _hoist_excess_waits(nc: bass.Bass):
    """Walrus encodes at most 1 semaphore wait on a DMA DIRECT2D / NoOp and 2
    on compute instruction structs.  Hoist excess waits onto NoOp instructions
    inserted just before the offender on the same engine."""
    import bass_rust
    nwh = 0
    for blk in nc.m.functions[0].blocks:
        insts = blk.instructions
        i = 0
        while i < len(insts):
            ins = insts[i]
            si = ins.sync_info
            limit = 1
            if si is not None and si.on_wait and len(si.on_wait) > limit:
                ow = list(si.on_wait)
                ins.sync_info = bass_rust.SyncInfo(
                    on_wait=[], on_update=list(si.on_update))
                pos = i
                for j in range(len(ow)):
                    nop = mybir.InstNoOp(name=f"WH{nwh}-{ins.name}",
                                         ins=[], outs=[])
                    nop.engine = ins.engine
                    nop.sync_info = bass_rust.SyncInfo(
                        on_wait=[ow[j]], on_update=[])
                    insts.insert(pos, nop)
                    pos += 1
                    nwh += 1
                i = pos + 1
            else:
                i += 1
    return nwh


def _build(os_imm: float) -> bass.Bass:
    nc = bass.Bass(trn_type="TRN2", num_swdge_queues=4, num_devices=NCORES)
    io = {
        "qT": nc.dram_tensor("qT", [E, LQ], I8, kind="ExternalInput"),
        "kh": nc.dram_tensor("kh", [E // 2, L], I8, kind="ExternalInput"),
        "vh": nc.dram_tensor("vh", [E // 2, L], I8, kind="ExternalInput"),
        "wq": nc.dram_tensor("wq", [E, 256], I8, kind="ExternalInput"),
        "wk": nc.dram_tensor("wk", [E, 256], I8, kind="ExternalInput"),
        "wv": nc.dram_tensor("wv", [E, 256], I8, kind="ExternalInput"),
        "wo": nc.dram_tensor("wo", [E, 256], I8, kind="ExternalInput"),
        "gq": nc.dram_tensor("gq", [1, LQ], F32, kind="ExternalInput"),
        "gk": nc.dram_tensor("gk", [1, L], F32, kind="ExternalInput"),
        "gv": nc.dram_tensor("gv", [P, TK], F32, kind="ExternalInput"),
        "qb": nc.dram_tensor("qb", [P, EC], F32, kind="ExternalInput"),
        "kb": nc.dram_tensor("kb", [P, EC], F32, kind="ExternalInput"),
        "vb": nc.dram_tensor("vb", [1, E], F16, kind="ExternalInput"),
        "ob": nc.dram_tensor("ob", [1, E], F32, kind="ExternalInput"),
        "oq": nc.dram_tensor("oq", [LQ, E], I8, kind="ExternalOutput"),
        "og": nc.dram_tensor("og", [P, TQ], F32, kind="ExternalOutput"),
    }
    io = {k: v[:] for k, v in io.items()}
    with ExitStack() as ctx:
        tc = ctx.enter_context(tile.TileContext(nc))
        _emit(ctx, tc, io, os_imm)
    _hoist_excess_waits(nc)
    nc.finalize()
    return nc


# ---------------------------------------------------------------- host ----

def _quant_act(x, scale):
    # x [B, L, E] f32; returns int8 [B, E, L] (transposed) and gamma*scale [B, L]
    g = jnp.maximum(jnp.max(jnp.abs(x), axis=-1), EPS)
    qx = jnp.clip(jnp.round(x * (QF / g)[..., None]), -QF, QF - 1.0)
    return jnp.swapaxes(qx, 1, 2).astype(jnp.int8), g * scale


def _q_prep(q, qs):
    qqT, gq = _quant_act(q, qs / (QF * SQRTD))
    cores = list(range(NCORES))
    qT_g = jnp.concatenate(
        [qqT[c // 2, :, (c % 2) * LQ:(c % 2 + 1) * LQ] for c in cores], 0)
    gq_g = jnp.stack([gq[c // 2, (c % 2) * LQ:(c % 2 + 1) * LQ] for c in cores], 0)
    return qT_g, gq_g


def _k_prep(k, ks):
    qkT, gk = _quant_act(k, ks / QF)
    kT_u = qkT.reshape(B * E, L)  # unique data: one copy per batch
    gk_g = jnp.stack([gk[c // 2] for c in range(NCORES)], 0)
    return kT_u, gk_g


def _v_prep(v, vs):
    qvT, gv = _quant_act(v, vs / QF)
    vT_u = qvT.reshape(B * E, L)
    gv_g = jnp.concatenate(
        [gv[c // 2].reshape(TK, P).T for c in range(NCORES)], 0)
    return vT_u, gv_g


def _quantize_weight(w):
    s = jnp.maximum(jnp.mean(jnp.abs(w)), EPS)
    qw = jnp.clip(jnp.round(w / s), -1.0, 1.0)
    return qw, s


def _pack_ternary(w):
    # w [E_out, E_in] ternary f32 -> packed int8 [E_in, 256] (base-27 along
    # e_out quarters of the transposed [E_in, E_out] matrix)
    wT = jnp.transpose(w).astype(jnp.int8)
    return (27 * wT[:, 0:256] + 9 * wT[:, 256:512]
            + 3 * wT[:, 512:768] + wT[:, 768:1024]).astype(jnp.int8)


def _weight_prep(ipw, ipb, opw, opb):
    qw_, kw_, vw_ = jnp.split(ipw, 3, 0)
    qb, kb, vb = jnp.split(ipb, 3, 0)
    qqw, qs = _quantize_weight(qw_)
    kqw, ks = _quantize_weight(kw_)
    vqw, vs = _quantize_weight(vw_)
    oqw, os_ = _quantize_weight(opw)
    packs = tuple(_pack_ternary(w) for w in (qqw, kqw, vqw, oqw))
    qbc = (qb / SQRTD).reshape(EC, P).T.astype(jnp.float32)
    kbc = kb.reshape(EC, P).T.astype(jnp.float32)
    vbr = vb[None, :].astype(jnp.float16)
    obr = opb[None, :].astype(jnp.float32)
    return packs, qbc, kbc, vbr, obr, qs, ks, vs, os_


# ---------------------------------------------------------------- runner ----

_CACHE: dict = {}
_WCACHE: dict = {}


def _io_layout(nc):
    in_names, out_names, out_avals = [], [], []
    for alloc in nc.m.functions[0].allocations:
        if not isinstance(alloc, mybir.MemoryLocationSet):
            continue
        name = alloc.memorylocations[0].name
        if alloc.kind == "ExternalInput":
            in_names.append(name)
        elif alloc.kind == "ExternalOutput":
            out_names.append(name)
            out_avals.append(jax.core.ShapedArray(
                tuple(alloc.tensor_shape), mybir.dt.np(alloc.dtype)))
    return in_names, out_names, out_avals


def _get_compiled(os_imm: float):
    key = round(float(os_imm), 12)
    if key in _CACHE:
        return _CACHE[key]
    install_neuronx_cc_hook()
    nc = _build(os_imm)
    in_names, out_names, out_avals = _io_layout(nc)
    part_name = (nc.partition_id_tensor.name
                 if nc.partition_id_tensor else None)
    if part_name is not None:
        in_names = [n for n in in_names if n != part_name]
    n_params = len(in_names)
    all_names = in_names + out_names
    if part_name is not None:
        all_names = all_names + [part_name]
    devices = jax.devices()[:NCORES]
    mesh = Mesh(np.asarray(devices).reshape(B, 2), ("pair", "half"))
    spec_core = PartitionSpec(("pair", "half"))
    spec_pair = PartitionSpec("pair")
    sh_core = NamedSharding(mesh, spec_core)
    sh_pair = NamedSharding(mesh, spec_pair)
    sh_half = sh_core  # kv halves shipped 8-way, resharded on device

    def _body(*args):
        operands = list(args)
        if part_name is not None:
            operands.append(partition_id_tensor())
        outs = _bass_exec_p.bind(
            *operands,
            out_avals=tuple(out_avals),
            in_names=tuple(all_names),
            out_names=tuple(out_names),
            lowering_input_output_aliases=(),
            sim_require_finite=True,
            sim_require_nnan=True,
            nc=nc,
        )
        return tuple(outs)

    donate = tuple(range(n_params, n_params + len(out_names)))
    in_specs = (spec_core,) * (n_params + len(out_names))
    out_specs = (spec_core,) * len(out_names)
    fn = jax.jit(
        shard_map(_body, mesh=mesh, in_specs=in_specs, out_specs=out_specs,
                  check_rep=False),
        donate_argnums=donate, keep_unused=True)

    zinfo = [(tuple(a.shape), a.dtype) for a in out_avals]

    def _mkzeros():
        return tuple(jnp.zeros((NCORES * s[0],) + s[1:], d) for s, d in zinfo)

    zeros_fn = jax.jit(_mkzeros,
                       out_shardings=tuple(sh_core for _ in zinfo))
    entry = (fn, zeros_fn, in_names, out_names, sh_core)
    _CACHE[key] = entry
    return entry


def _fingerprint(*arrs):
    return tuple((a.shape, str(a.dtype), zlib.adler32(a.tobytes()))
                 for a in arrs)


_CPU = None


def _cpu():
    global _CPU
    if _CPU is None:
        _CPU = jax.devices("cpu")[0]
    return _CPU


def _run(inputs, trace=False, **_ignored):
    cpu = _cpu()
    ipw = np.asarray(inputs["in_proj_weight"], np.float32)
    ipb = np.asarray(inputs["in_proj_bias"], np.float32)
    opw = np.asarray(inputs["out_proj_weight"], np.float32)
    opb = np.asarray(inputs["out_proj_bias"], np.float32)

    wkey = _fingerprint(ipw, ipb, opw, opb)
    if wkey not in _WCACHE:
        with jax.default_device(cpu):
            wjit = jax.jit(_weight_prep)
            packs, qbc, kbc, vbr, obr, qs, ks, vs, os_ = wjit(
                jax.device_put(ipw, cpu), jax.device_put(ipb, cpu),
                jax.device_put(opw, cpu), jax.device_put(opb, cpu))
            packs = [np.asarray(p) for p in packs]
            qbc, kbc = np.asarray(qbc), np.asarray(kbc)
            vbr, obr = np.asarray(vbr), np.asarray(obr)
            qs, ks, vs, os_ = (float(qs), float(ks), float(vs), float(os_))
        fn, zeros_fn, in_names, out_names, sh_core = _get_compiled(os_)
        # weight-derived global arrays, committed to device once
        wdev = {
            "wq": jax.device_put(np.tile(packs[0], (NCORES, 1)), sh_core),
            "wk": jax.device_put(np.tile(packs[1], (NCORES, 1)), sh_core),
            "wv": jax.device_put(np.tile(packs[2], (NCORES, 1)), sh_core),
            "wo": jax.device_put(np.tile(packs[3], (NCORES, 1)), sh_core),
            "qb": jax.device_put(np.tile(qbc, (NCORES, 1)), sh_core),
            "kb": jax.device_put(np.tile(kbc, (NCORES, 1)), sh_core),
            "vb": jax.device_put(np.tile(vbr, (NCORES, 1)), sh_core),
            "ob": jax.device_put(np.tile(obr, (NCORES, 1)), sh_core),
        }
        for a in wdev.values():
            a.block_until_ready()
        _WCACHE[wkey] = (wdev, qs, ks, vs, os_)
    wdev, qs, ks, vs, os_ = _WCACHE[wkey]
    fn, zeros_fn, in_names, out_names, sh_core = _get_compiled(os_)

    zeros = zeros_fn()  # on-device, async
    query = np.asarray(inputs["query"], np.float32)
    key = np.asarray(inputs["key"], np.float32)
    value = np.asarray(inputs["value"], np.float32)
    # pipeline: per-tensor cpu quant -> async shard transfer.  k/v ship as
    # 8-way half-shards (unique bytes only); the bass kernel itself runs a
    # pair AllGather to reassemble full kT/vT on device.
    with jax.default_device(cpu):
        kjit = _CACHE.setdefault("_kjit", jax.jit(_k_prep))
        vjit = _CACHE.setdefault("_vjit", jax.jit(_v_prep))
        qjit = _CACHE.setdefault("_qjit", jax.jit(_q_prep))
        kT_u, gk_g = kjit(key, jnp.float32(ks))
        kh_d = jax.device_put(kT_u, sh_core)
        vT_u, gv_g = vjit(value, jnp.float32(vs))
        vh_d = jax.device_put(vT_u, sh_core)
        qT_g, gq_g = qjit(query, jnp.float32(qs))
        qT_d = jax.device_put(qT_g, sh_core)
    acts = {
        "qT": qT_d, "kh": kh_d, "vh": vh_d,
        "gq": np.asarray(gq_g), "gk": np.asarray(gk_g),
        "gv": np.asarray(gv_g),
    }
    args = [wdev[n] if n in wdev else acts[n] for n in in_names]
    outs = fn(*args, *zeros)
    out_map = {n: np.asarray(o) for n, o in zip(out_names, outs)}

    oq = out_map["oq"].reshape(NCORES, LQ, E).astype(np.float32)
    og = out_map["og"].reshape(NCORES, P, TQ)
    out = np.empty((B, L, E), np.float32)
    for c in range(NCORES):
        bb, qh = c // 2, c % 2
        gscale = (og[c].T.reshape(LQ) / OQ).astype(np.float32)
        out[bb, qh * LQ:(qh + 1) * LQ, :] = oq[c] * gscale[:, None]

    class _Res:
        exec_time_ns = None

    return out, _Res()


def kernel(**inputs) -> np.ndarray:
    out, _ = _run(inputs)
    return out
